# revision 8
# baseline (speedup 1.0000x reference)
"""GATNet Trainium kernel: bf16 sharded GAT with fused node/agg phases.

Design (8-way dst-shard of nodes, SPMD on 8 single-core axon terminals):
  Round r (0..4) produces the layer-r node table T(r)=[h|es] in bf16,
  window by window (49 windows of 128 dst nodes per core). For r>=1 each
  window fuses the aggregation of layer r-1 with the next layer's node
  matmul: per-edge gather of T rows ([128,1]-offset indirect DMAs), dst
  scores scattered to edge slots via a host-baked one-hot S_T matmul,
  edge softmax weights on ACT/DVE, segment-sum via one-hot S matmuls into
  PSUM, epilogue (normalize + bias + relu), PE transpose, and the node
  matmul producing T(r) - no DRAM roundtrip of activations. T shards are
  exchanged once per round with a bf16 AllGather (half the bytes of f32;
  the emulated collective here costs ~per-byte). ed scores stay core-local
  in parity-buffered tables read directly per window (no gather).
  The final aggregation feeds the graph mean-pool matmuls straight from
  SBUF; pool partials go through one small AllReduce; the tiny MLP runs
  replicated. All matmul operands are bf16 (4x PE throughput), PSUM f32.
"""
import sys
sys.path.insert(0, "/opt/trn_rl_repo")
sys.path.insert(0, "/opt/trn_rl_repo/concourse")
import numpy as np
import ml_dtypes
from concourse import bass, bacc, mybir
import concourse.tile as tile
import bass_rust
from bass_rust import add_dep_helper

BF16 = mybir.dt.bfloat16
F32 = mybir.dt.float32
I32 = mybir.dt.int32
AF = mybir.ActivationFunctionType

N = 50000
E = 400000
G = 256
H = 2
C = 100
HC = 200
F_IN = 336
SLOPE = 0.2
NCORE = 8
SHARD = N // NCORE            # 6250
P = 128
SHARD_PAD = 6272              # 49*128
NW = 49                       # windows per core
ROWB = 256                    # padded T row (bf16 -> 512B)
NFULL_PAD = NCORE * SHARD_PAD
CH = 7                        # windows per exchange chunk
NCHUNK = NW // CH             # 7 chunks per round
CHCOLS = CH * ROWB            # bf16 cols per chunk slice


def prep(x, edge_index, batch, Ws, asrcs, adsts, bcs, lws, lbs):
    bf = ml_dtypes.bfloat16
    src_all = np.concatenate([edge_index[0], np.arange(N, dtype=np.int64)])
    dst_all = np.concatenate([edge_index[1], np.arange(N, dtype=np.int64)])
    order = np.argsort(dst_all, kind="stable")
    s_sorted = src_all[order].astype(np.int64)
    d_sorted = dst_all[order].astype(np.int64)

    win_edges = []
    smax = 1
    for k in range(NCORE):
        for w in range(NW):
            d_lo = k * SHARD + w * P
            d_hi = min(k * SHARD + min((w + 1) * P, SHARD), N)
            lo = np.searchsorted(d_sorted, d_lo, side="left")
            hi = np.searchsorted(d_sorted, d_hi, side="left")
            win_edges.append((lo, hi))
            smax = max(smax, -(-(hi - lo) // P))
    SMAX = smax
    SLOTS = SMAX * P

    per_core = []
    for k in range(NCORE):
        gd = np.zeros((NW, P, SMAX), np.int32)
        sblob = np.zeros((NW, P, SLOTS), bf)
        stblob = np.zeros((NW, P, SLOTS), bf)
        for w in range(NW):
            lo, hi = win_edges[k * NW + w]
            ne = hi - lo
            srcs = np.zeros(SLOTS, np.int64)
            dloc = np.zeros(SLOTS, np.int64)
            if ne > 0:
                srcs[:ne] = s_sorted[lo:hi]
                srcs[ne:] = s_sorted[lo]
                dloc[:ne] = d_sorted[lo:hi] - (k * SHARD + w * P)
            sl = np.arange(SLOTS)
            c_src = srcs // SHARD
            r_src = srcs % SHARD
            trow = c_src * SHARD_PAD + r_src
            gd[w, sl % P, sl // P] = trow
            S = np.zeros((SLOTS, P), np.float32)
            if ne > 0:
                S[np.arange(ne), dloc[:ne]] = 1.0
            # S blob: [slot=(j,p), m] -> [p, (j m)]
            sblob[w] = S.reshape(SMAX, P, P).transpose(1, 0, 2).reshape(P, SLOTS).astype(bf)
            # S_T blob: [m, (j p)] as lhsT tiles [128 m, 128 slot] per j
            stblob[w] = S.reshape(SMAX, P, P).transpose(2, 0, 1).reshape(P, SLOTS).astype(bf)
        # xT0: [3, 128, SHARD_PAD] bf16 feature-major
        xT0 = np.zeros((3, P, SHARD_PAD), bf)
        xT = x[k * SHARD:(k + 1) * SHARD].T.astype(np.float32)  # [336, 6250]
        for c in range(3):
            rows = xT[c * P:min((c + 1) * P, F_IN)]
            xT0[c, :rows.shape[0], :SHARD] = rows.astype(bf)
        # pool mask [49, 128, 256] bf16
        pmask = np.zeros((NW, P, G), bf)
        bsh = batch[k * SHARD:(k + 1) * SHARD]
        for t in range(NW):
            r0 = t * P
            r1 = min(r0 + P, SHARD)
            if r1 > r0:
                pmask[t, np.arange(r1 - r0), bsh[r0:r1]] = 1.0
        per_core.append(dict(xT0=xT0, gd=gd, sblob=sblob, stblob=stblob, pmask=pmask))

    # shared weights
    waug = np.zeros((11, P, 204), bf)
    biasrep = np.zeros((5, P, HC), np.float32)
    ci = 0
    for li in range(5):
        W = Ws[li].astype(np.float32)
        a_s, a_d = asrcs[li].astype(np.float32), adsts[li].astype(np.float32)
        was = np.zeros((W.shape[0], 2), np.float32)
        wad = np.zeros((W.shape[0], 2), np.float32)
        for h in range(H):
            was[:, h] = W[:, h * C:(h + 1) * C] @ a_s[h]
            wad[:, h] = W[:, h * C:(h + 1) * C] @ a_d[h]
        aug = np.concatenate([W, was, wad], axis=1)  # [F, 204]
        nch = 3 if li == 0 else 2
        for c in range(nch):
            rows = aug[c * P:(c + 1) * P]
            waug[ci, :rows.shape[0]] = rows.astype(bf)
            ci += 1
        biasrep[li, :, :] = bcs[li].astype(np.float32)
    assert ci == 11

    ident = np.eye(P, dtype=np.float32)
    mlpb = np.zeros((3, P, 1), np.float32)
    mlpb[0, :100, 0] = lbs[0]
    mlpb[1, :100, 0] = lbs[1]
    mlpb[2, :29, 0] = lbs[2]
    shared = dict(waug=waug, biasrep=biasrep, ident=ident,
                  mlw1=lws[0].astype(np.float32), mlw2=lws[1].astype(np.float32),
                  mlw3=lws[2].astype(np.float32), mlpb=mlpb)
    return per_core, shared, dict(SMAX=SMAX)


def build_nc(SMAX, no_ag=False):
    nc = bacc.Bacc("TRN2", target_bir_lowering=False)

    # inputs
    xT0 = nc.declare_dram_parameter("xT0", [3, P, SHARD_PAD], BF16, isOutput=False)
    gd = nc.declare_dram_parameter("gd", [NW, P, SMAX], I32, isOutput=False)
    SLOTS = SMAX * P
    sblob = nc.declare_dram_parameter("sblob", [NW, P, SLOTS], BF16, isOutput=False)
    stblob = nc.declare_dram_parameter("stblob", [NW, P, SLOTS], BF16, isOutput=False)
    pmask = nc.declare_dram_parameter("pmask", [NW, P, G], BF16, isOutput=False)
    waug = nc.declare_dram_parameter("waug", [11, P, 204], BF16, isOutput=False)
    biasrep = nc.declare_dram_parameter("biasrep", [5, P, HC], F32, isOutput=False)
    ident_in = nc.declare_dram_parameter("ident", [P, P], F32, isOutput=False)
    mlw1 = nc.declare_dram_parameter("mlw1", [200, 100], F32, isOutput=False)
    mlw2 = nc.declare_dram_parameter("mlw2", [100, 100], F32, isOutput=False)
    mlw3 = nc.declare_dram_parameter("mlw3", [100, 29], F32, isOutput=False)
    mlpb = nc.declare_dram_parameter("mlpb", [3, P, 1], F32, isOutput=False)
    out = nc.declare_dram_parameter("out", [29, G], F32, isOutput=True)

    # internal DRAM
    TC_ = 202
    Tshard = nc.dram_tensor("Tshard", [SHARD_PAD, TC_], BF16)
    Tfull = nc.dram_tensor("Tfull", [NFULL_PAD, TC_], BF16, addr_space="Shared")
    ed_tab = [nc.dram_tensor(f"ed_tab{i}", [SHARD_PAD, 2], BF16) for i in range(2)]
    cc2_in = nc.dram_tensor("cc2_in", [201, G], F32)
    cc2_out = nc.dram_tensor("cc2_out", [201, G], F32, addr_space="Shared")
    rg = [list(range(NCORE))]

    with tile.TileContext(nc) as tc:
        with tc.tile_pool(name="const", bufs=1) as cpool:
            wtiles = []
            for i in range(11):
                wt = cpool.tile([P, 204], BF16, tag=f"waug{i}")
                nc.sync.dma_start(out=wt[:], in_=waug[i])
                wtiles.append(wt)
            ident = cpool.tile([P, P], F32, tag="ident")
            nc.sync.dma_start(out=ident[:], in_=ident_in[:])
            ones1f = cpool.tile([1, P], F32, tag="ones1f")
            nc.vector.memset(ones1f[:], 1.0)
            brt = cpool.tile([P, HC], F32, tag="brt")

            with tc.tile_pool(name="sb", bufs=3) as pool, \
                 tc.tile_pool(name="sb3", bufs=4) as pool3:
              with tc.tile_pool(name="ps", bufs=2, space="PSUM") as pspool:

                def node_tile(r, w, lhsTs):
                    """Node matmuls producing T(r) window w from lhsT tiles."""
                    ci0 = 0 if r == 0 else 3 + 2 * (r - 1)
                    npsum = pspool.tile([P, 204], F32, tag="npsum")
                    nck = len(lhsTs)
                    for c, (lt, kdim) in enumerate(lhsTs):
                        nc.tensor.matmul(
                            out=npsum[:], lhsT=lt[0:kdim, :], rhs=wtiles[ci0 + c][0:kdim, :],
                            start=(c == 0), stop=(c == nck - 1))
                    Tt = pool.tile([P, TC_], BF16, tag="Tt")
                    nc.vector.tensor_copy(out=Tt[:], in_=npsum[:, 0:TC_])
                    nc.sync.dma_start(out=Tshard[w * P:(w + 1) * P, :], in_=Tt[:])
                    edt = pool.tile([P, 2], BF16, tag="edt")
                    nc.vector.tensor_copy(out=edt[:], in_=npsum[:, 202:204])
                    nc.sync.dma_start(
                        out=ed_tab[r % 2][w * P:(w + 1) * P, :], in_=edt[:])

                def agg_window(la, w):
                    """Aggregate layer la, window w -> stb [P, 200] f32 relu'd."""
                    ED = ed_tab[la % 2]
                    gdt = pool3.tile([P, SMAX], I32, tag="gdt")
                    nc.sync.dma_start(out=gdt[:], in_=gd[w])
                    Gt = pool.tile([P, SMAX, TC_], BF16, tag="Gt")
                    for j in range(SMAX):
                        nc.gpsimd.indirect_dma_start(
                            out=Gt[:, j, :], out_offset=None, in_=Tfull[:],
                            in_offset=bass.IndirectOffsetOnAxis(ap=gdt[:, j:j + 1], axis=0))
                    st = pool.tile([P, SLOTS], BF16, tag="st")
                    nc.sync.dma_start(out=st[:], in_=sblob[w])
                    stt = pool.tile([P, SLOTS], BF16, tag="stt")
                    nc.sync.dma_start(out=stt[:], in_=stblob[w])
                    edw = pool.tile([P, 2], BF16, tag="edw")
                    nc.sync.dma_start(out=edw[:], in_=ED[w * P:(w + 1) * P, :])
                    apsum = pspool.tile([P, 202 + 2 * SMAX], F32, tag="apsum")
                    for j in range(SMAX):
                        nc.tensor.matmul(
                            out=apsum[:, 202 + 2 * j:202 + 2 * j + 2],
                            lhsT=stt[:, j * P:(j + 1) * P],
                            rhs=edw[:], start=True, stop=True)
                    e1 = pool.tile([P, SMAX, 2], F32, tag="e1")
                    nc.vector.tensor_add(
                        out=e1[:], in0=Gt[:, :, 200:202],
                        in1=apsum[:, 202:202 + 2 * SMAX].rearrange("p (j c) -> p j c", c=2))
                    e2 = pool.tile([P, SMAX, 2], F32, tag="e2")
                    nc.vector.tensor_scalar_mul(out=e2[:], in0=e1[:], scalar1=SLOPE)
                    e3 = pool.tile([P, SMAX, 2], F32, tag="e3")
                    nc.vector.tensor_tensor(out=e3[:], in0=e1[:], in1=e2[:],
                                            op=mybir.AluOpType.max)
                    wv = pool.tile([P, SMAX, 2], F32, tag="wv")
                    nc.scalar.activation(out=wv[:], in_=e3[:], func=AF.Exp)
                    Vt = pool.tile([P, SMAX, TC_], BF16, tag="Vt")
                    nc.vector.tensor_mul(
                        out=Vt[:, :, 0:200].rearrange("p j (h c) -> p j h c", h=H),
                        in0=Gt[:, :, 0:200].rearrange("p j (h c) -> p j h c", h=H),
                        in1=wv[:].unsqueeze(3).broadcast_to([P, SMAX, 2, C]))
                    nc.vector.tensor_copy(out=Vt[:, :, 200:202], in_=wv[:])
                    for j in range(SMAX):
                        nc.tensor.matmul(
                            out=apsum[:, 0:TC_], lhsT=st[:, j * P:(j + 1) * P],
                            rhs=Vt[:, j, :],
                            start=(j == 0), stop=(j == SMAX - 1))
                    sc = pool.tile([P, 2], F32, tag="sc")
                    nc.vector.tensor_scalar_add(out=sc[:], in0=apsum[:, 200:202],
                                                scalar1=1e-30)
                    rc = pool.tile([P, 2], F32, tag="rc")
                    nc.vector.reciprocal(out=rc[:], in_=sc[:])
                    stg = pool.tile([P, HC], F32, tag="stg")
                    nc.vector.tensor_scalar_mul(out=stg[:, 0:100], in0=apsum[:, 0:100],
                                                scalar1=rc[:, 0:1])
                    nc.vector.tensor_scalar_mul(out=stg[:, 100:200], in0=apsum[:, 100:200],
                                                scalar1=rc[:, 1:2])
                    nc.vector.tensor_add(out=stg[:], in0=stg[:], in1=brt[:])
                    stb = pool.tile([P, HC], F32, tag="stb")
                    nc.scalar.activation(out=stb[:], in_=stg[:], func=AF.Relu)
                    return stb

                def do_allgather():
                    tc.strict_bb_all_engine_barrier()
                    if not no_ag:
                        nc.gpsimd.collective_compute(
                            "AllGather", mybir.AluOpType.bypass, replica_groups=rg,
                            ins=[Tshard[:]], outs=[Tfull[:]])
                    tc.strict_bb_all_engine_barrier()

                # ---- round 0: node phase from xT0 ----
                for w in range(NW):
                    lts = []
                    for c in range(3):
                        lt = pool3.tile([P, P], BF16, tag="nlhsT")
                        nc.sync.dma_start(out=lt[:], in_=xT0[c][:, w * P:(w + 1) * P])
                        lts.append((lt, P))
                    node_tile(0, w, lts)
                do_allgather()

                # ---- rounds 1..4: fused agg(r-1) + node(r) ----
                for r in range(1, 5):
                    la = r - 1
                    nc.sync.dma_start(out=brt[:], in_=biasrep[la])
                    for w in range(NW):
                        stb = agg_window(la, w)
                        tp1 = pspool.tile([P, P], F32, tag="tp")
                        nc.tensor.transpose(out=tp1[:], in_=stb[:, 0:128], identity=ident[:])
                        tp2 = pspool.tile([P, P], F32, tag="tp")
                        nc.tensor.transpose(out=tp2[0:72, :], in_=stb[:, 128:200],
                                            identity=ident[:])
                        tr1 = pool.tile([P, P], BF16, tag="tr1")
                        nc.vector.tensor_copy(out=tr1[:], in_=tp1[:])
                        tr2 = pool.tile([72, P], BF16, tag="tr2")
                        nc.vector.tensor_copy(out=tr2[:], in_=tp2[0:72, :])
                        node_tile(r, w, [(tr1, P), (tr2, 72)])
                    do_allgather()

                # ---- final aggregation (layer 4) + pool matmuls ----
                nc.sync.dma_start(out=brt[:], in_=biasrep[4])
                with tc.tile_pool(name="ps2", bufs=1, space="PSUM") as ps2:
                    ppA = ps2.tile([P, G], F32, tag="ppA")
                    ppB = ps2.tile([73, G], F32, tag="ppB")
                    for w in range(NW):
                        stb = agg_window(4, w)
                        sxb1 = pool.tile([P, P], BF16, tag="sxb1")
                        nc.vector.tensor_copy(out=sxb1[:], in_=stb[:, 0:128])
                        sxb2 = pool.tile([P, 73], BF16, tag="sxb2")
                        nc.vector.tensor_copy(out=sxb2[:, 0:72], in_=stb[:, 128:200])
                        nc.vector.memset(sxb2[:, 72:73], 1.0)
                        pmt = pool3.tile([P, G], BF16, tag="pmt")
                        nc.sync.dma_start(out=pmt[:], in_=pmask[w])
                        nc.tensor.matmul(out=ppA[:], lhsT=sxb1[:], rhs=pmt[:],
                                         start=(w == 0), stop=(w == NW - 1))
                        nc.tensor.matmul(out=ppB[:], lhsT=sxb2[:], rhs=pmt[:],
                                         start=(w == 0), stop=(w == NW - 1))
                    pstA = pool.tile([P, G], F32, tag="pstA")
                    nc.vector.tensor_copy(out=pstA[:], in_=ppA[:])
                    pstB = pool.tile([73, G], F32, tag="pstB")
                    nc.vector.tensor_copy(out=pstB[:], in_=ppB[:])
                    nc.sync.dma_start(out=cc2_in[0:128, :], in_=pstA[:])
                    nc.sync.dma_start(out=cc2_in[128:201, :], in_=pstB[:])

              # ---- pool allreduce + MLP tail ----
              tc.strict_bb_all_engine_barrier()
              nc.gpsimd.collective_compute(
                  "AllReduce", mybir.AluOpType.add, replica_groups=rg,
                  ins=[cc2_in[:]], outs=[cc2_out[:]])
              tc.strict_bb_all_engine_barrier()
              with tc.tile_pool(name="ps3", bufs=1, space="PSUM") as ps3:
                plA = pool.tile([P, G], F32, tag="plA")
                nc.sync.dma_start(out=plA[:], in_=cc2_out[0:128, :])
                plB = pool.tile([73, G], F32, tag="plB")
                nc.sync.dma_start(out=plB[:], in_=cc2_out[128:201, :])
                cntr = pool.tile([1, G], F32, tag="cntr")
                nc.sync.dma_start(out=cntr[:], in_=cc2_out[200:201, :])
                cntm = pool.tile([1, G], F32, tag="cntm")
                nc.vector.tensor_scalar_max(out=cntm[:], in0=cntr[:], scalar1=1.0)
                rc2 = pool.tile([1, G], F32, tag="rc2")
                nc.vector.reciprocal(out=rc2[:], in_=cntm[:])
                Rb = ps3.tile([P, G], F32, tag="Rb")
                nc.tensor.matmul(out=Rb[:], lhsT=ones1f[:], rhs=rc2[:], start=True, stop=True)
                mA = pool.tile([P, G], F32, tag="mA")
                nc.vector.tensor_mul(out=mA[:], in0=plA[:], in1=Rb[:])
                mB = pool.tile([72, G], F32, tag="mB")
                nc.vector.tensor_mul(out=mB[:], in0=plB[0:72, :], in1=Rb[0:72, :])
                w1a = pool.tile([P, 100], F32, tag="w1a")
                nc.sync.dma_start(out=w1a[:], in_=mlw1[0:128, :])
                w1b = pool.tile([72, 100], F32, tag="w1b")
                nc.sync.dma_start(out=w1b[:], in_=mlw1[128:200, :])
                w2t = pool.tile([100, 100], F32, tag="w2t")
                nc.sync.dma_start(out=w2t[:], in_=mlw2[:])
                w3t = pool.tile([100, 29], F32, tag="w3t")
                nc.sync.dma_start(out=w3t[:], in_=mlw3[:])
                b1 = pool.tile([P, 1], F32, tag="b1")
                nc.sync.dma_start(out=b1[:], in_=mlpb[0])
                b2 = pool.tile([P, 1], F32, tag="b2")
                nc.sync.dma_start(out=b2[:], in_=mlpb[1])
                b3 = pool.tile([P, 1], F32, tag="b3")
                nc.sync.dma_start(out=b3[:], in_=mlpb[2])
                y1p = ps3.tile([100, G], F32, tag="y1p")
                nc.tensor.matmul(out=y1p[:], lhsT=w1a[:], rhs=mA[:], start=True, stop=False)
                nc.tensor.matmul(out=y1p[:], lhsT=w1b[:], rhs=mB[:], start=False, stop=True)
                y1 = pool.tile([100, G], F32, tag="y1")
                nc.scalar.activation(out=y1[:], in_=y1p[:], func=AF.Relu, bias=b1[0:100, :])
                y2p = ps3.tile([100, G], F32, tag="y2p")
                nc.tensor.matmul(out=y2p[:], lhsT=w2t[:], rhs=y1[:], start=True, stop=True)
                y2 = pool.tile([100, G], F32, tag="y2")
                nc.scalar.activation(out=y2[:], in_=y2p[:], func=AF.Relu, bias=b2[0:100, :])
                y3p = ps3.tile([29, G], F32, tag="y3p")
                nc.tensor.matmul(out=y3p[:], lhsT=w3t[:], rhs=y2[:], start=True, stop=True)
                y3 = pool.tile([29, G], F32, tag="y3")
                nc.scalar.activation(out=y3[:], in_=y3p[:], func=AF.Identity, bias=b3[0:29, :])
                nc.sync.dma_start(out=out[:], in_=y3[:])

    nc.finalize()
    return nc


def make_in_maps(per_core, shared):
    return [{**pc, **shared} for pc in per_core]


# ---------------- runner (device-resident SPMD invoke) ----------------
import jax
from jax.sharding import Mesh, PartitionSpec, NamedSharding
from jax.experimental.shard_map import shard_map
from concourse import bass2jax
from concourse.bass2jax import _bass_exec_p, install_neuronx_cc_hook, partition_id_tensor


class SpmdRunner:
    def __init__(self, nc, n_cores=8):
        install_neuronx_cc_hook()
        self.nc = nc
        self.n_cores = n_cores
        partition_name = nc.partition_id_tensor.name if nc.partition_id_tensor else None
        in_names, out_names, out_avals, zero_outs = [], [], [], []
        for alloc in nc.m.functions[0].allocations:
            if not isinstance(alloc, mybir.MemoryLocationSet):
                continue
            name = alloc.memorylocations[0].name
            if alloc.kind == "ExternalInput":
                if name != partition_name and name != (nc.dbg_addr.name if nc.dbg_addr else None):
                    in_names.append(name)
            elif alloc.kind == "ExternalOutput":
                out_names.append(name)
                shape = tuple(alloc.tensor_shape)
                dtype = mybir.dt.np(alloc.dtype)
                out_avals.append(jax.core.ShapedArray(shape, dtype))
                zero_outs.append(np.zeros(shape, dtype))
        self.in_names, self.out_names = in_names, out_names
        self.out_avals, self.zero_outs = out_avals, zero_outs
        n_params, n_outs = len(in_names), len(out_names)
        self.n_params = n_params
        all_in_names = list(in_names) + list(out_names)
        if nc.dbg_addr is not None:
            all_in_names.append(nc.dbg_addr.name)
        if partition_name is not None:
            all_in_names.append(partition_name)
        self.has_dbg = nc.dbg_addr is not None

        def _body(*args):
            operands = list(args)
            if self.has_dbg:
                operands.append(jax.numpy.zeros((1, 2), jax.numpy.uint32))
            if partition_name is not None:
                operands.append(partition_id_tensor())
            outs = _bass_exec_p.bind(
                *operands,
                out_avals=tuple(out_avals),
                in_names=tuple(all_in_names),
                out_names=tuple(out_names),
                lowering_input_output_aliases=(),
                sim_require_finite=False,
                sim_require_nnan=False,
                nc=nc,
            )
            return tuple(outs)

        devices = jax.devices()[:n_cores]
        self.mesh = Mesh(np.asarray(devices), ("core",))
        in_specs = (PartitionSpec("core"),) * (n_params + n_outs)
        out_specs = (PartitionSpec("core"),) * n_outs
        donate = tuple(range(n_params, n_params + n_outs))
        self.sharded = jax.jit(
            shard_map(_body, mesh=self.mesh, in_specs=in_specs,
                      out_specs=out_specs, check_rep=False),
            donate_argnums=donate, keep_unused=True,
        )
        self.sharding = NamedSharding(self.mesh, PartitionSpec("core"))
        self.dev_in = None

    def stage_inputs(self, in_maps):
        per_core = [[np.asarray(m[n]) for n in self.in_names] for m in in_maps]
        concat_in = [
            np.concatenate([per_core[c][i] for c in range(self.n_cores)], axis=0)
            for i in range(self.n_params)
        ]
        self.dev_in = [jax.device_put(a, self.sharding) for a in concat_in]
        for a in self.dev_in:
            a.block_until_ready()

    def __call__(self):
        concat_zeros = [
            jax.device_put(
                np.zeros((self.n_cores * z.shape[0], *z.shape[1:]), z.dtype),
                self.sharding)
            for z in self.zero_outs
        ]
        out = self.sharded(*self.dev_in, *concat_zeros)
        for o in out:
            o.block_until_ready()
        return out

    def results(self, out):
        return [
            {
                name: np.asarray(out[i]).reshape(self.n_cores, *self.out_avals[i].shape)[c]
                for i, name in enumerate(self.out_names)
            }
            for c in range(self.n_cores)
        ]


# ---------------- entry point ----------------
_CACHE = {}


def _get_runner(SMAX):
    if SMAX not in _CACHE:
        nc = build_nc(SMAX)
        _CACHE[SMAX] = SpmdRunner(nc, NCORE)
    return _CACHE[SMAX]


def kernel(**inputs):
    x = np.asarray(inputs["x"], np.float32)
    edge_index = np.asarray(inputs["edge_index"])
    batch = np.asarray(inputs["batch"])
    Ws = [np.asarray(inputs[f"W{i+1}"], np.float32) for i in range(5)]
    asrcs = [np.asarray(inputs[f"asrc{i+1}"], np.float32) for i in range(5)]
    adsts = [np.asarray(inputs[f"adst{i+1}"], np.float32) for i in range(5)]
    bcs = [np.asarray(inputs[f"bc{i+1}"], np.float32) for i in range(5)]
    lws = [np.asarray(inputs[f"lw{i+1}"], np.float32) for i in range(3)]
    lbs = [np.asarray(inputs[f"lb{i+1}"], np.float32) for i in range(3)]
    per_core, shared, meta = prep(x, edge_index, batch, Ws, asrcs, adsts, bcs, lws, lbs)
    r = _get_runner(meta["SMAX"])
    r.stage_inputs(make_in_maps(per_core, shared))
    out = r()
    y3T = r.results(out)[0]["out"]      # [29, 256] f32
    return np.ascontiguousarray(y3T.T)  # [256, 29]
